# revision 29
# baseline (speedup 1.0000x reference)
"""Trainium2 Bass kernel for nn_CausalBankModel (decay-bank LM head), v8.

HW-measured ~368-372us (vs 492899 ns bf16 baseline, ~1.34x), rel_err
1.12e-2 (gate 2e-2). The tensor stream is GAPLESS (100% dense, trace-
verified): 325us of it is the 1536-instr readout sweep, which is the
information-theoretic floor for this precision split (fp8 DoubleRow
carries 2x MACs/cycle but e4m3 pairs cost ~2.3e-2 noise per 256-dim pair
on the lin branch - only the small-magnitude loc branch can absorb fp8).
Remaining overhead is ~8.5us fixed NEFF preamble + ~5.5us output drain.
Head/tail reorderings beyond this config measured WORSE (early tensor
gaps also reset the PE p-state ramp, ~+2.5us each) - keep the DMA issue
order, engine assignments, and 14-instr warm-up as-is.

Sharding (8 NeuronCores): DP4 x TP2.
  core c -> token group tg = c//2 (batch b = tg//2, half hb = tg%2 -> 512
  tokens), vocab half vh = c%2 (16000 cols).

v6 precision split (sim rel_err 1.12e-2 vs the 2e-2 gate): the loc branch's
logits are ~6x smaller than lin's (x embeddings have sigma 0.02 vs the decay
-bank states' ~0.07+), so its quantization noise is discounted ~6x in the
g~=0.5 blend. The ENTIRE loc pipeline therefore runs in fp8 e4m3 with
MatmulPerfMode.DoubleRow (2x MACs/cycle: 256-dim contraction per instr at
1 cyc/row, HW-measured 210.8ns per 500-col matmul):
  local hidden  relu(win(x)@lw1): x*512 and lw1*512 in e4m3, 8 DROW instr
  loc readout   h2@lw2: h2*256 and lw2*128 in e4m3, 4 DROW per (ti,chunk)
The lin branch (dominant magnitude) stays fully bf16 - one fp8 pair there
alone costs 2.3e-2 (fails). Readout: 12 instr/(ti,chunk) vs 16 all-bf16.

The gather/unshard on the host: per-position stats over the full vocab ->
sigmoid gate -> blend (g*lin + (1-g)*loc) in fp32 numpy.

Layouts (partition dim first):
  xtb  [128(d%128), 2(d//128), 1031] bf16, 7 zero cols of causal pad; this
       core's 512 tokens at cols 519..1030. xtb8 = e4m3(xtb*512), cast on
       device (scalar engine) under the mode-projection matmuls.
  hT   [128(hid%128), 8(hid//128), 512(tok)] bf16  - lin readout lhsT.
  h28  [128, 8, 512] e4m3 (*256)                   - loc readout lhsT.
  w2b  [128, 32(chunk), 8, 500] bf16   - chunk-major: 8000B DMA lines.
  lw28 [128, 32(chunk), 8, 500] e4m3 (*128): 4000B lines.
  lin_d/loc_d [128(tok%128), 4(tile), 16000] bf16 - streamed logit outputs.
"""

import os
import sys

import numpy as np

for _p in ("/opt/trn_rl_repo", "/opt/pypackages"):
    if _p not in sys.path and os.path.isdir(_p):
        sys.path.append(_p)

import ml_dtypes  # noqa: E402

from concourse import bacc, bass, tile  # noqa: E402
from concourse import mybir  # noqa: E402
from concourse.bass_utils import run_bass_kernel_spmd  # noqa: E402

F32 = mybir.dt.float32
BF16 = mybir.dt.bfloat16
FP8 = mybir.dt.float8e4
ALU = mybir.AluOpType
ACTF = mybir.ActivationFunctionType
DROW = mybir.MatmulPerfMode.DoubleRow

V = 32000
D = 256
M = 256
W = 8
HL = 1024
B = 2
S = 1024
NCORE = 8
ST = 512              # tokens per core
NT = ST // 128        # 4 token tiles
VS = V // 2           # 16000 vocab cols per core
CW = 500              # chunk width (32*500 = 16000 exactly)
NVC = VS // CW        # 32 chunks
SP = S + W - 1        # 1031 padded time length
T0 = SP - ST          # 519: first col of this core's tokens
SX = 512.0            # e4m3 scale for x and lw1 (local hidden operands)
SH2 = 256.0           # e4m3 scale for h2 (loc readout lhsT)
SW2 = 128.0           # e4m3 scale for lw2
# loc PSUM carries SH2*SW2 * logits; local-hidden PSUM carries SX^2 * preact
LOC_DSC = 1.0 / (SH2 * SW2)
LH_SC = SH2 / (SX * SX)

LAST_RESULT = None


def build(nc, with_vocab_bias):
    din = {}

    def inp(name, shape, dt):
        din[name] = nc.dram_tensor(name, list(shape), dt, kind="ExternalInput")
        return din[name]

    xtb_d = inp("xtb", [128, 2 * SP], BF16)
    inprojb_d = inp("inprojb", [128, 2, M], BF16)
    decb_d = inp("decb", [128, 2, 512], F32)
    w1b_d = inp("w1b", [128, 4, HL], BF16)
    b1r_d = inp("b1r", [128, HL // 128], F32)
    lw1b8_d = inp("lw1b8", [128, 16, HL], FP8)
    lb1r8_d = inp("lb1r8", [128, HL // 128], F32)
    w2b_d = inp("w2b", [128, NVC, 8, CW], BF16)
    lw28_d = inp("lw28", [128, NVC, 8, CW], FP8)
    if with_vocab_bias:
        ones_d = inp("ones", [1, 128], BF16)
        b2_d = inp("b2", [1, VS], BF16)
        lb2_d = inp("lb2", [1, VS], BF16)

    lin_d = nc.dram_tensor("lin", [128, NT, VS], BF16, kind="ExternalOutput")
    loc_d = nc.dram_tensor("loc", [128, NT, VS], BF16, kind="ExternalOutput")

    with tile.TileContext(nc) as tc:
        with (
            tc.tile_pool(name="cst", bufs=1) as cst,
            tc.tile_pool(name="ps", bufs=8, space=bass.MemorySpace.PSUM) as psp,
        ):
            # Short PE warm-up: the HAM clock gate holds the array at 1.2GHz
            # until ~3.4us of sustained activity; a dozen dummy matmuls on a
            # zeroed tile finish before the first input DMA lands, so real
            # trunk matmuls start at 2.4GHz.
            # 14 (not 12): the modes matmuls un-gate at ~14.3us (input DMA);
            # the warm-up must bridge that exactly - a tensor gap there both
            # idles the PE and resets the p-state ramp (measured +2.5us per
            # early gap).
            warm = cst.tile([128, 640], BF16)
            nc.vector.memset(warm[:], 0.0)
            for wi in range(14):
                wps = psp.tile([128, 512], F32, tag="ps", name="ps")
                nc.tensor.matmul(wps[:], warm[:, 0:128], warm[:, 128:640],
                                 start=True, stop=True)

            b1r_sb = cst.tile([128, 8], F32)
            lb1r8_sb = cst.tile([128, 8], F32)
            if with_vocab_bias:
                ones_sb = cst.tile([1, 128], BF16)
                nc.sync.dma_start(ones_sb[:], ones_d[:, :])

            # weight/slab pools allocated OUTSIDE the trunk scratch pool so
            # their SBUF ranges don't alias it: weight prefetch then streams
            # during the trunk instead of waiting for its last reader (WAR).
            with (
                tc.tile_pool(name="wst", bufs=2) as wst,
                tc.tile_pool(name="slab", bufs=3) as slp,
                tc.tile_pool(name="ph", bufs=1) as php,
            ):
                hT = php.tile([128, 8, ST], BF16)
                h28 = php.tile([128, 8, ST], FP8)

                # ---------------- trunk ----------------
                with tc.tile_pool(name="pa", bufs=1) as pap:
                    # load order: modes matmuls (hf=0) read xtb cols 7..519,
                    # so load those first; lw1b8 next (local hidden), then
                    # scan/linear inputs.
                    xtb = pap.tile([128, 2, SP], BF16)
                    xtb8 = pap.tile([128, 2, SP], FP8)
                    lw1b8_sb = pap.tile([128, 16, HL], FP8)
                    inprojb_sb = pap.tile([128, 2, M], BF16)
                    # first wave split across the two HWDGE queues so the
                    # modes matmuls (needing xtb heads + inprojb) un-gate
                    # before the PE warm-up drains. (Measured-best ordering:
                    # fewer-piece or rebalanced variants both regressed by
                    # opening early tensor gaps, which also reset the PE
                    # p-state ramp.)
                    HM = W - 1 + 512
                    for dh in range(2):
                        nc.sync.dma_start(xtb[:, dh, 0:HM],
                                          xtb_d[:, dh * SP:dh * SP + HM])
                    nc.scalar.dma_start(inprojb_sb[:], inprojb_d[:, :, :])
                    for dh in range(2):
                        nc.scalar.dma_start(xtb[:, dh, HM:SP],
                                            xtb_d[:, dh * SP + HM:(dh + 1) * SP])
                    for q in range(8):
                        nc.sync.dma_start(lw1b8_sb[:, q * 2:(q + 1) * 2, :],
                                          lw1b8_d[:, q * 2:(q + 1) * 2, :])
                    nc.sync.dma_start(b1r_sb[:], b1r_d[:, :])
                    nc.scalar.dma_start(lb1r8_sb[:], lb1r8_d[:, :])
                    decb_sb = pap.tile([128, 2, 512], F32)
                    nc.sync.dma_start(decb_sb[:], decb_d[:, :, :])
                    w1b_sb = pap.tile([128, 4, HL], BF16)
                    for q in range(2):
                        nc.sync.dma_start(w1b_sb[:, q * 2:(q + 1) * 2, :],
                                          w1b_d[:, q * 2:(q + 1) * 2, :])

                    # fp8 cast of x for the local-hidden DROW matmuls; runs
                    # on the (idle) vector engine under the modes matmuls so
                    # the scalar engine's queue stays clear for its DMAs.
                    for dh in range(2):
                        nc.vector.tensor_scalar_mul(xtb8[:, dh, :],
                                                    xtb[:, dh, :], SX)

                    statesT = pap.tile([128, 2, S], F32)
                    statesb = pap.tile([128, 2, 512], BF16)

                    # mode projection + decay scan over the full 1024-slot
                    # prefix (zero prefix for first-half cores)
                    for mt in range(2):
                        for hf in range(2):
                            ps = psp.tile([128, 512], F32)
                            for kt in range(2):
                                nc.tensor.matmul(
                                    ps[:],
                                    inprojb_sb[:, kt, mt * 128:(mt + 1) * 128],
                                    xtb[:, kt, W - 1 + hf * 512:W - 1 + hf * 512 + 512],
                                    start=(kt == 0), stop=(kt == 1),
                                )
                            init = (0.0 if hf == 0 else
                                    statesT[:, mt, hf * 512 - 1:hf * 512])
                            nc.vector.tensor_tensor_scan(
                                statesT[:, mt, hf * 512:hf * 512 + 512],
                                decb_sb[:, mt, :], ps[:], init,
                                ALU.mult, ALU.add,
                            )
                        nc.vector.tensor_copy(statesb[:, mt, :],
                                              statesT[:, mt, 512:1024])

                    # local-window hidden in fp8 DoubleRow: 8 instrs, each
                    # contracting the (window i, d-half 0/1) pair. lw1b8's
                    # dim1 is ki = i*2+dh, and xtb8's dim1 is dh, so the
                    # rhs pair slice lines up with lw1b8[:, 2i:2i+2, :].
                    lps = [psp.tile([128, 512], F32, name=f"lps{hl}", tag="ps")
                           for hl in range(8)]
                    for i in range(8):
                        rhs = xtb8[:, :, T0 - W + 1 + i:T0 - W + 1 + i + ST]
                        for hl in range(8):
                            nc.tensor.matmul(
                                lps[hl][:],
                                lw1b8_sb[:, 2 * i:2 * i + 2,
                                         hl * 128:(hl + 1) * 128],
                                rhs, start=(i == 0), stop=(i == 7),
                                perf_mode=DROW,
                            )
                    for hl in range(8):
                        nc.scalar.activation(h28[:, hl, :], lps[hl][:],
                                             ACTF.Relu, scale=LH_SC,
                                             bias=lb1r8_sb[:, hl:hl + 1])

                    # linear-readout hidden (bf16): feat = [states|x]
                    for hl in range(8):
                        ps = psp.tile([128, 512], F32)
                        for kt in range(4):
                            if kt < 2:
                                rhs = statesb[:, kt, :]
                            else:
                                rhs = xtb[:, kt - 2, T0:T0 + ST]
                            nc.tensor.matmul(
                                ps[:], w1b_sb[:, kt, hl * 128:(hl + 1) * 128],
                                rhs, start=(kt == 0), stop=(kt == 3),
                            )
                        nc.scalar.activation(hT[:, hl, :], ps[:], ACTF.Relu,
                                             bias=b1r_sb[:, hl:hl + 1])

                # ---------------- readout sweep ----------------
                # per-chunk slabs, loc (fp8 DROW, ready first) then lin
                # (bf16); slab DMAs stream under the next chunk's matmuls.
                for vc in range(NVC):
                    w2t = wst.tile([128, 8, CW], BF16, name="w2t", tag="w2t")
                    nc.sync.dma_start(w2t[:], w2b_d[:, vc, :, :])
                    lw2t = wst.tile([128, 8, CW], FP8, name="lw2t", tag="lw2t")
                    nc.sync.dma_start(lw2t[:], lw28_d[:, vc, :, :])
                    bias_sb = [None, None]
                    if with_vocab_bias:
                        for br, bd in enumerate((b2_d, lb2_d)):
                            bt = wst.tile([1, CW], BF16, name=f"bt{br}",
                                          tag=f"bt{br}")
                            nc.sync.dma_start(bt[:],
                                              bd[:, vc * CW:(vc + 1) * CW])
                            bias_sb[br] = bt
                    slabs = [slp.tile([128, NT, CW], BF16,
                                      name=f"slab{br}", tag=f"slab{br}")
                             for br in range(2)]
                    for br in (1, 0):
                        for ti in range(NT):
                            ps = psp.tile([128, CW], F32)
                            if with_vocab_bias:
                                nc.tensor.matmul(
                                    ps[:], ones_sb[:, :], bias_sb[br][:],
                                    start=True, stop=False)
                            if br == 1:
                                for kt in range(4):
                                    nc.tensor.matmul(
                                        ps[:],
                                        h28[:, 2 * kt:2 * kt + 2,
                                            ti * 128:(ti + 1) * 128],
                                        lw2t[:, 2 * kt:2 * kt + 2, :],
                                        start=(kt == 0 and not with_vocab_bias),
                                        stop=(kt == 3),
                                        perf_mode=DROW,
                                    )
                                nc.scalar.activation(
                                    slabs[1][:, ti, :], ps[:],
                                    ACTF.Copy, scale=LOC_DSC)
                            else:
                                for kt in range(8):
                                    nc.tensor.matmul(
                                        ps[:],
                                        hT[:, kt, ti * 128:(ti + 1) * 128],
                                        w2t[:, kt, :],
                                        start=(kt == 0 and not with_vocab_bias),
                                        stop=(kt == 7),
                                    )
                                if vc == NVC - 1 and ti == NT - 1:
                                    # the very last PSUM copy is tail-exposed:
                                    # split it so the first half's DMA flies
                                    # under the second half's copy.
                                    HW2 = CW // 2
                                    nc.scalar.activation(
                                        slabs[0][:, ti, 0:HW2],
                                        ps[:, 0:HW2], ACTF.Copy)
                                    nc.scalar.activation(
                                        slabs[0][:, ti, HW2:CW],
                                        ps[:, HW2:CW], ACTF.Copy)
                                else:
                                    nc.scalar.activation(
                                        slabs[0][:, ti, :], ps[:], ACTF.Copy)
                    v0 = vc * CW
                    nc.scalar.dma_start(loc_d[:, :, v0:v0 + CW], slabs[1][:])
                    if vc < NVC - 1:
                        nc.sync.dma_start(lin_d[:, :, v0:v0 + CW],
                                          slabs[0][:])
                    else:
                        # tail: split the final lin slab per-ti so the DMAs
                        # overlap the last PSUM copies; the final ti goes out
                        # in two halves (first half under the second half's
                        # copy), last piece on the lower-latency HWDGE queue.
                        HW2 = CW // 2
                        for ti in range(NT):
                            if ti < NT - 1:
                                eng = nc.gpsimd if ti < 2 else nc.sync
                                eng.dma_start(lin_d[:, ti, v0:v0 + CW],
                                              slabs[0][:, ti, :])
                            else:
                                nc.gpsimd.dma_start(
                                    lin_d[:, ti, v0:v0 + HW2],
                                    slabs[0][:, ti, 0:HW2])
                                nc.sync.dma_start(
                                    lin_d[:, ti, v0 + HW2:v0 + CW],
                                    slabs[0][:, ti, HW2:CW])

    nc.compile()
    return din, (lin_d, loc_d)


_CACHED = {}


def _get_program(with_vocab_bias):
    if with_vocab_bias not in _CACHED:
        nc = bacc.Bacc("TRN2", target_bir_lowering=False, debug=False,
                       num_devices=NCORE)
        build(nc, with_vocab_bias=with_vocab_bias)
        _CACHED[with_vocab_bias] = nc
    return _CACHED[with_vocab_bias]


def _prep_inputs(tokens, emb, in_proj, decays, w1, b1, w2, b2,
                 lw1, lb1, lw2, lb2, gate_w, gate_b, with_vocab_bias):
    BF = ml_dtypes.bfloat16
    F8 = ml_dtypes.float8_e4m3
    tokens = np.asarray(tokens).astype(np.int64)          # [2,1024]
    emb = np.asarray(emb, np.float32)
    x = emb[tokens]                                       # [2,1024,256]

    inprojb = np.ascontiguousarray(
        np.asarray(in_proj, np.float32).reshape(2, 128, M)
        .transpose(1, 0, 2)).astype(BF)
    decays = np.asarray(decays, np.float32)
    decb = np.ascontiguousarray(
        np.broadcast_to(decays.reshape(2, 128).transpose(1, 0)[:, :, None],
                        (128, 2, 512))).astype(np.float32)
    w1b = np.ascontiguousarray(
        np.asarray(w1, np.float32).reshape(4, 128, HL)
        .transpose(1, 0, 2)).astype(BF)
    lw1b8 = np.ascontiguousarray(
        (np.asarray(lw1, np.float32) * np.float32(SX)).reshape(8, 2, 128, HL)
        .transpose(2, 0, 1, 3).reshape(128, 16, HL)).astype(F8)
    b1r = np.ascontiguousarray(
        np.asarray(b1, np.float32).reshape(8, 128).T)
    lb1r8 = np.ascontiguousarray(
        np.asarray(lb1, np.float32).reshape(8, 128).T) * np.float32(SH2)

    shared = {"inprojb": inprojb, "decb": decb, "w1b": w1b, "b1r": b1r,
              "lw1b8": lw1b8, "lb1r8": lb1r8}
    if with_vocab_bias:
        shared["ones"] = np.ones((1, 128), BF)

    def chunk_major(w, scale, dt):
        # [HL, V] -> per-half [128, NVC, 8, CW]
        wr = (np.asarray(w, np.float32) * np.float32(scale)
              ).reshape(8, 128, V).transpose(1, 0, 2)
        out = []
        for vh in range(2):
            half = wr[:, :, vh * VS:(vh + 1) * VS]
            half = half.reshape(128, 8, NVC, CW).transpose(0, 2, 1, 3)
            out.append(np.ascontiguousarray(half).astype(dt))
        return out

    w2v = chunk_major(w2, 1.0, BF)
    lw2v = chunk_major(lw2, SW2, F8)
    wv = []
    for vh in range(2):
        sl = slice(vh * VS, (vh + 1) * VS)
        e = {"w2b": w2v[vh], "lw28": lw2v[vh]}
        if with_vocab_bias:
            e["b2"] = np.asarray(b2, np.float32)[sl].reshape(1, VS).astype(BF)
            e["lb2"] = (np.asarray(lb2, np.float32)[sl]
                        * np.float32(SH2 * SW2)).reshape(1, VS).astype(BF)
        wv.append(e)

    xg = []
    for tg in range(4):
        b, hb = tg // 2, tg % 2
        xt = np.zeros((128, 2, SP), np.float32)
        if hb == 0:
            for dh in range(2):
                xt[:, dh, T0:] = x[b, 0:ST, dh * 128:(dh + 1) * 128].T
        else:
            for dh in range(2):
                xt[:, dh, W - 1:] = x[b, :, dh * 128:(dh + 1) * 128].T
        xg.append(np.ascontiguousarray(xt.reshape(128, 2 * SP)).astype(BF))

    in_maps = []
    for c in range(NCORE):
        tg, vh = c // 2, c % 2
        m = dict(shared)
        m["xtb"] = xg[tg]
        m.update(wv[vh])
        in_maps.append(m)
    return in_maps


def kernel(**inputs):
    global LAST_RESULT
    with_vocab_bias = bool(np.any(np.asarray(inputs["b2"]))
                           or np.any(np.asarray(inputs["lb2"])))
    nc = _get_program(with_vocab_bias)
    in_maps = _prep_inputs(**inputs, with_vocab_bias=with_vocab_bias)
    res = run_bass_kernel_spmd(nc, in_maps, list(range(NCORE)))
    LAST_RESULT = res

    # gather/unshard + gated mixture of the per-core logit shards
    lin = np.empty((B, S, V), np.float32)
    loc = np.empty((B, S, V), np.float32)
    for c in range(NCORE):
        tg, vh = c // 2, c % 2
        b, hb = tg // 2, tg % 2
        ts, vsl = slice(hb * ST, (hb + 1) * ST), slice(vh * VS, (vh + 1) * VS)
        # lin/loc device layout: [128(tok%128), 4(tile), VS]
        lin[b, ts, vsl] = (res.results[c]["lin"].astype(np.float32)
                           .transpose(1, 0, 2).reshape(ST, VS))
        loc[b, ts, vsl] = (res.results[c]["loc"].astype(np.float32)
                           .transpose(1, 0, 2).reshape(ST, VS))

    gate_w = np.asarray(inputs["gate_w"], np.float32).reshape(6)
    gate_b = np.asarray(inputs["gate_b"], np.float32).reshape(1)

    def stats(z):
        m = z.mean(-1)
        sd = z.std(-1)
        mx = z.max(-1)
        return m, mx, sd

    ml_, xl, sl_ = stats(lin)
    mc, xc, sc = stats(loc)
    zarg = (gate_w[0] * ml_ + gate_w[1] * xl + gate_w[2] * sl_
            + gate_w[3] * mc + gate_w[4] * xc + gate_w[5] * sc + gate_b[0])
    g = (1.0 / (1.0 + np.exp(-zarg)))[..., None]
    return g * lin + (1.0 - g) * loc
